# revision 60
# baseline (speedup 1.0000x reference)
"""Additive (Bahdanau) attention scoring kernel for Trainium2, 8-core SPMD.

Reference computation (B=16, S=4096, D=1024, all fp32):
    q      = target @ Wq.T                    # [B, D]
    k      = memory @ Wk.T                    # [B, S, D]
    scores = tanh(q[:, None, :] + k) @ v      # [B, S]
    out    = softmax(scores - 1e9 * mask, axis=-1)

Sharding: 2 batches per core (host-paired: largest-kept with
smallest-kept), weights replicated.

Host-side prep is layout/quantize only: memory is transposed to [D, S]
per batch, compacted to the unmasked positions (masked positions
contribute exactly 0 to the softmax since exp(-1e9) underflows to 0 in
fp32, so dropping them is algebraically exact), cast to bf16, and
pre-tiled so each DMA is one contiguous block. Both batches of a core
are PACKED into one compact position stream: slot0's kept positions
padded to a fixed boundary BD (the global max kept count), slot1's
appended right after — so the core processes ceil((BD+n1)/128) s-tiles
of 128 positions instead of 2*ceil(BD/128). Exactly one tile (MIXJ)
straddles the batch boundary; a constant 0/1 selector builds its mixed
q-tile, and per-batch masks split the two softmax reductions.

Device layout ("s on partitions"): for each s-tile the PE computes
  k_ps[s=128, e=1024] += memtile[d=128, s=128].T @ WkT[d=128, e]
accumulated over the 8 d-chunks (16 matmuls of N=512, mem stationary,
Wk moving, all bf16). Everything else runs off the PE: DVE adds the
batch's q-tile (materialized once via K=2 selector matmuls against the
on-device q = Wq-matmul result), ACT applies tanh (fp32 PSUM -> bf16
SBUF), and a DVE scalar_tensor_tensor multiplies by the broadcast v
with accum_out (per-partition free-axis sum) yielding the 128 scores.
Finale: add a 0/-1e9 pad-penalty, ACT Exp, two masked DVE reductions
(one per batch), two 128x128 ones-matmuls to reduce+broadcast the
totals, then one fused scale pass and a single compact DMA out. The
host scatters to full S (pure indexing; masked positions are exactly
0).
"""

import math
import os
from contextlib import ExitStack

import ml_dtypes
import numpy as np

import concourse.tile as tile
from concourse import bacc, mybir
import concourse.bass as bass  # noqa: F401

B, S, D = 16, 4096, 1024
N_CORES = 8
NB = B // N_CORES  # batches per core
P = 128
DC = D // P        # contraction chunks
EH = D // 512      # moving-operand halves (PSUM bank = 512 fp32)

F32 = mybir.dt.float32
BF16 = mybir.dt.bfloat16
AF = mybir.ActivationFunctionType
MUL = mybir.AluOpType.mult
ADD = mybir.AluOpType.add

BF16NP = ml_dtypes.bfloat16

_CACHE = {}


def _chunks(T):
    """DMA chunks of up to 4 s-tiles (1 MiB of bf16 per full chunk). The
    first chunk is a single tile so the PE's first matmul dependency is a
    256 KiB transfer."""
    return [(0, 1)] + [(i, min(4, T - i)) for i in range(1, T, 4)]


def _build_program(T, mixj, stage):
    chunks = _chunks(T)

    nc = bacc.Bacc("TRN2", target_bir_lowering=False, debug=False)

    memC = nc.dram_tensor("memC", [P, DC * T * P], BF16, kind="ExternalInput").ap()
    wkT = nc.dram_tensor("wkT", [P, DC * D], BF16, kind="ExternalInput").ap()
    wqT = nc.dram_tensor("wqT", [P, DC * D], BF16, kind="ExternalInput").ap()
    tgtT = nc.dram_tensor("tgtT", [P, DC * NB], BF16, kind="ExternalInput").ap()
    vbc = nc.dram_tensor("vbc", [P, D], BF16, kind="ExternalInput").ap()
    pen = nc.dram_tensor("pen", [P, T], F32, kind="ExternalInput").ap()
    bm0 = nc.dram_tensor("bm0", [P, T], F32, kind="ExternalInput").ap()
    bm1 = nc.dram_tensor("bm1", [P, T], F32, kind="ExternalInput").ap()
    sel = nc.dram_tensor("sel", [NB, 3 * P], F32, kind="ExternalInput").ap()
    # head0 = [Wk d-chunk 0 | memC tile 0] packed so one completion wait
    # gates the first real matmul
    head0 = nc.dram_tensor("head0", [P, 2 * D], BF16, kind="ExternalInput").ap()
    out = nc.dram_tensor("out", [P, T], F32, kind="ExternalOutput").ap()

    with tile.TileContext(nc) as tc, ExitStack() as ctx:
        consts = ctx.enter_context(tc.tile_pool(name="consts", bufs=1))
        mem_pool = ctx.enter_context(tc.tile_pool(name="mem", bufs=3))
        ti_pool = ctx.enter_context(tc.tile_pool(name="ti", bufs=3))
        tt_pool = ctx.enter_context(tc.tile_pool(name="tt", bufs=3))
        ttv_pool = ctx.enter_context(tc.tile_pool(name="ttv", bufs=2))
        fin_pool = ctx.enter_context(tc.tile_pool(name="fin", bufs=2))
        kps_pool = ctx.enter_context(tc.tile_pool(name="kps", bufs=3, space="PSUM"))
        qps_pool = ctx.enter_context(tc.tile_pool(name="qps", bufs=1, space="PSUM"))

        # DMA plumbing: the per-DMA completion cost serializes per HWDGE
        # ring, so head-critical transfers are split across both rings.
        head_sb = consts.tile([P, 2 * D], BF16)
        nc.scalar.dma_start(head_sb[:], head0[:, :])
        w1 = chunks[1][1] * P
        mem1_sb = mem_pool.tile([P, DC * 4 * P], BF16, tag="mem", name="mem_sb")
        nc.scalar.dma_start(
            mem1_sb[:, :DC * w1],
            memC[:, DC * chunks[1][0] * P: DC * chunks[1][0] * P + DC * w1],
        )
        # Wk chunks pace the lead k-matmuls on sync's ring; Wq follows them
        # on the SAME ring so later memC chunk DMAs (emitted in-loop, hence
        # behind Wq in this ring's FIFO) cannot steal its HBM bandwidth —
        # the q-matmuls stalled ~2.8us on late Wq when it rode the other
        # ring against the memC stream.
        wk_sb = consts.tile([P, DC * D], BF16)
        for dc in range(DC):
            nc.sync.dma_start(
                wk_sb[:, dc * D:(dc + 1) * D], wkT[:, dc * D:(dc + 1) * D]
            )
        wq_sb = consts.tile([P, DC * D], BF16)
        nc.sync.dma_start(wq_sb[:], wqT[:, :])
        tgt_sb = consts.tile([P, DC * NB], BF16)
        nc.scalar.dma_start(tgt_sb[:], tgtT[:, :])
        sel_sb = consts.tile([NB, 3 * P], F32)
        nc.scalar.dma_start(sel_sb[:], sel[:, :])
        vbc_sb = consts.tile([P, D], BF16)
        nc.scalar.dma_start(vbc_sb[:], vbc[:, :])
        pen_sb = consts.tile([P, T], F32)
        nc.scalar.dma_start(pen_sb[:], pen[:, :])
        bm0_sb = consts.tile([P, T], F32)
        nc.scalar.dma_start(bm0_sb[:], bm0[:, :])
        bm1_sb = consts.tile([P, T], F32)
        nc.scalar.dma_start(bm1_sb[:], bm1[:, :])

        ones128 = consts.tile([P, P], F32)
        nc.vector.memset(ones128[:], 1.0)

        # PE pre-warm with VARYING data (HAM watches switching activity, not
        # busy cycles — all-zero dummies left it cold, and identical-operand
        # dummies stop toggling after the first pass). Two iota tiles and a
        # rotating stationary slice give every dummy matmul fresh products,
        # so the clock gate reaches 8/8 before the real k-matmuls arrive.
        io_sb = consts.tile([P, 512], F32)
        nc.gpsimd.iota(io_sb[:], pattern=[[1, 512]], base=1, channel_multiplier=3,
                       allow_small_or_imprecise_dtypes=True)
        io2_sb = consts.tile([P, 512], F32)
        nc.gpsimd.iota(io2_sb[:], pattern=[[2, 512]], base=7, channel_multiplier=11,
                       allow_small_or_imprecise_dtypes=True)
        warm_ps = qps_pool.tile([P, D], F32, tag="qps", name="warm_ps")
        for i in range(8):
            lhs = (io_sb if i % 2 == 0 else io2_sb)[:, (i % 4) * P:(i % 4 + 1) * P]
            rhs = io2_sb if i % 2 == 0 else io_sb
            nc.tensor.matmul(warm_ps[:, 0:512], lhs, rhs[:], start=True, stop=True)

        q_sb = consts.tile([NB, D], F32)
        qt_sb = consts.tile([P, 3 * D], F32)
        score_sb = consts.tile([P, T], F32)
        rs_sbs = [consts.tile([P, 1], F32, tag=f"rs{i}", name=f"rs{i}") for i in range(2)]
        rc_sbs = [consts.tile([P, 1], F32, tag=f"rc{i}", name=f"rc{i}") for i in range(2)]

        def emit_qsetup():
            # q = target @ Wq.T on the PE, fp32 accumulate -> [NB, D]
            q_ps = qps_pool.tile([P, D], F32, tag="qps", name="q_ps")
            for eh in range(EH):
                for dc in range(DC):
                    nc.tensor.matmul(
                        q_ps[0:NB, eh * 512:(eh + 1) * 512],
                        tgt_sb[:, dc * NB:(dc + 1) * NB],
                        wq_sb[:, dc * D + eh * 512: dc * D + (eh + 1) * 512],
                        start=(dc == 0),
                        stop=(dc == DC - 1),
                    )
            nc.vector.tensor_copy(q_sb[:], q_ps[0:NB, :])
            # q-tiles (slot0 / slot1 / mixed boundary tile) via K=2 selector
            # matmuls: qt[i][s, e] = sum_b sel[b, i*P+s] * q[b, e]
            for i in range(3):
                qt_ps = qps_pool.tile([P, D], F32, tag="qps", name="qt_ps")
                for eh in range(EH):
                    nc.tensor.matmul(
                        qt_ps[:, eh * 512:(eh + 1) * 512],
                        sel_sb[:, i * P:(i + 1) * P],
                        q_sb[0:NB, eh * 512:(eh + 1) * 512],
                        start=True,
                        stop=True,
                    )
                nc.vector.tensor_copy(qt_sb[:, i * D:(i + 1) * D], qt_ps[:])

        def qt_slice(j):
            i = 0 if j < mixj else (2 if j == mixj else 1)
            return qt_sb[:, i * D:(i + 1) * D]

        def emit_tile0_mm():
            # tile 0 reads both its stationary (mem tile 0) and the dc=0
            # moving chunk from the packed head0 transfer. eh-outer order:
            # 8 consecutive matmuls per PSUM bank instead of alternating
            # banks every matmul (bank cycling triggers HAM oscillation).
            k_ps = kps_pool.tile([P, D], F32, tag="kps", name="k_ps")
            for dc in range(DC):
                for eh in range(EH):
                    mv = (head_sb[:, eh * 512:(eh + 1) * 512] if dc == 0
                          else wk_sb[:, dc * D + eh * 512: dc * D + (eh + 1) * 512])
                    nc.tensor.matmul(
                        k_ps[:, eh * 512:(eh + 1) * 512],
                        head_sb[:, D + dc * P: D + (dc + 1) * P],
                        mv,
                        start=(dc == 0),
                        stop=(dc == DC - 1),
                    )
            return k_ps

        def emit_tile_mm(mem_sb, w, t):
            k_ps = kps_pool.tile([P, D], F32, tag="kps", name="k_ps")
            for dc in range(DC):
                for eh in range(EH):
                    nc.tensor.matmul(
                        k_ps[:, eh * 512:(eh + 1) * 512],
                        mem_sb[:, dc * w + t * P: dc * w + (t + 1) * P],
                        wk_sb[:, dc * D + eh * 512: dc * D + (eh + 1) * 512],
                        start=(dc == 0),
                        stop=(dc == DC - 1),
                    )
            return k_ps

        def emit_tile_post(k_ps, j, split=False):
            ti = ti_pool.tile([P, D], F32, tag="ti", name="ti")
            tt = tt_pool.tile([P, D], BF16, tag="tt", name="tt")
            ttv = ttv_pool.tile([P, D], BF16, tag="ttv", name="ttv")
            if not split:
                nc.vector.tensor_add(ti[:], k_ps[:], qt_slice(j))
                nc.scalar.activation(tt[:], ti[:], AF.Tanh)
                nc.vector.scalar_tensor_tensor(
                    ttv[:], tt[:], 1.0, vbc_sb[:],
                    op0=MUL, op1=MUL,
                    accum_out=score_sb[:, j:j + 1],
                )
                return
            # tail latency: run the last tile's post chain in two pipelined
            # halves so the end-of-kernel serial chain is shorter
            qs = qt_slice(j)
            sc = consts.tile([P, 2], F32, tag="scsplit", name="scsplit")
            for h in range(2):
                hs = slice(h * 512, (h + 1) * 512)
                nc.vector.tensor_add(ti[:, hs], k_ps[:, hs], qs[:, hs])
                nc.scalar.activation(tt[:, hs], ti[:, hs], AF.Tanh)
                nc.vector.scalar_tensor_tensor(
                    ttv[:, hs], tt[:, hs], 1.0, vbc_sb[:, hs],
                    op0=MUL, op1=MUL,
                    accum_out=sc[:, h:h + 1],
                )
            nc.vector.tensor_add(score_sb[:, j:j + 1], sc[:, 0:1], sc[:, 1:2])

        # --- main loop over the packed s-tiles. The first 3 tiles'
        # k-matmuls (= kps pool depth) are emitted before the q-setup so
        # the PE can start as soon as head0 lands; their post-processing
        # (which needs the q-tiles) follows it. slot0's exp + masked
        # reduction (bm0 is zero past the boundary tile) runs mid-stream
        # right after the boundary tile, keeping the end-of-kernel chain
        # to slot1 only. ---
        sm = consts.tile([P, T], F32)
        ex = consts.tile([P, T], F32)
        c0 = mixj + 1  # columns 0:c0 cover every bm0-nonzero position
        ems = [fin_pool.tile([P, T], F32, tag=f"em{i}", name=f"em{i}") for i in range(2)]

        st0 = fin_pool.tile([P, T], F32, tag="st0", name="st0")

        def emit_fin0():
            nc.vector.tensor_add(sm[:, 0:c0], score_sb[:, 0:c0], pen_sb[:, 0:c0])
            if stage >= 2:
                nc.scalar.activation(ex[:, 0:c0], sm[:, 0:c0], AF.Exp)
                nc.vector.scalar_tensor_tensor(
                    ems[0][:, 0:c0], ex[:, 0:c0], 1.0, bm0_sb[:, 0:c0],
                    op0=MUL, op1=MUL, accum_out=rs_sbs[0][:],
                )

        def emit_fin0_back():
            # slot0's totals matmul + scale + its slice of the output DMA,
            # emitted a few tiles after fin0 so the PE has dependency slack
            # — only slot1's chain remains after the last k-matmul
            tot_ps = qps_pool.tile([P, D], F32, tag="qps", name="tot_ps")
            nc.tensor.matmul(tot_ps[:, 0:1], ones128[:], rs_sbs[0][:], start=True, stop=True)
            nc.vector.reciprocal(rc_sbs[0][:], tot_ps[:, 0:1])
            nc.vector.tensor_scalar_mul(st0[:, 0:c0], ems[0][:, 0:c0], rc_sbs[0][:])
            nc.sync.dma_start(out[:, 0:mixj], st0[:, 0:mixj])

        done_fin0 = False
        for ci, (coff, cnt) in enumerate(chunks):
            w = cnt * P
            if ci == 0:
                lead0 = emit_tile0_mm()
                continue
            if ci == 1:
                mem_sb = mem1_sb
            else:
                mem_sb = mem_pool.tile([P, DC * 4 * P], BF16, tag="mem", name="mem_sb")
                nc.sync.dma_start(
                    mem_sb[:, :DC * w],
                    memC[:, DC * coff * P: DC * coff * P + DC * w],
                )
            if ci == 1:
                leads = [emit_tile_mm(mem_sb, w, t) for t in range(2)]
                emit_qsetup()
                emit_tile_post(lead0, 0)
                for t in range(2):
                    emit_tile_post(leads[t], coff + t)
                for t in range(2, cnt):
                    emit_tile_post(emit_tile_mm(mem_sb, w, t), coff + t)
                continue
            last = ci == len(chunks) - 1
            for t in range(cnt):
                j = coff + t
                emit_tile_post(
                    emit_tile_mm(mem_sb, w, t), j,
                    split=(last and t == cnt - 1),
                )
                if j == mixj:
                    emit_fin0()
                    done_fin0 = True
                if j == mixj + 3 and stage >= 2:
                    emit_fin0_back()
        if not done_fin0:
            emit_fin0()
            if stage >= 2:
                emit_fin0_back()

        # --- finale back half: slot1's chain + the shared scale/output ---
        nc.vector.tensor_add(sm[:, c0:T], score_sb[:, c0:T], pen_sb[:, c0:T])
        if stage < 2:
            nc.sync.dma_start(out[:, :], sm[:])
        else:
            nc.scalar.activation(ex[:, c0:T], sm[:, c0:T], AF.Exp)
            nc.vector.scalar_tensor_tensor(
                ems[1][:, mixj:T], ex[:, mixj:T], 1.0, bm1_sb[:, mixj:T],
                op0=MUL, op1=MUL, accum_out=rs_sbs[1][:],
            )
            tot_ps = qps_pool.tile([P, D], F32, tag="qps", name="tot_ps")
            nc.tensor.matmul(tot_ps[:, 0:1], ones128[:], rs_sbs[1][:], start=True, stop=True)
            nc.vector.reciprocal(rc_sbs[1][:], tot_ps[:, 0:1])
            # out = em0*r0 + em1*r1 (em_i are already the masked exps);
            # em0 is defined on cols 0:c0, em1 on cols mixj:T — only the
            # boundary column mixj needs both terms. Cols 0:mixj went out
            # with emit_fin0_back already.
            ot = fin_pool.tile([P, T], F32, tag="ot", name="ot")
            nc.vector.scalar_tensor_tensor(
                ot[:, mixj:c0], ems[1][:, mixj:c0], rc_sbs[1][:], st0[:, mixj:c0],
                op0=MUL, op1=ADD,
            )
            nc.vector.tensor_scalar_mul(ot[:, c0:T], ems[1][:, c0:T], rc_sbs[1][:])
            nc.sync.dma_start(out[:, mixj:T], ot[:, mixj:T])

    nc.compile()
    return nc


def get_program(T=None, mixj=None, stage=None):
    if stage is None:
        stage = int(os.environ.get("KERNEL_STAGE", "2"))
    key = (T, mixj, stage)
    if key not in _CACHE:
        _CACHE[key] = _build_program(T, mixj, stage)
    return _CACHE[key]


def prepare_in_maps(memory, target, memory_mask, Wq, Wk, v):
    memory = np.asarray(memory, dtype=np.float32)
    target = np.asarray(target, dtype=np.float32)
    Wq = np.asarray(Wq, dtype=np.float32)
    Wk = np.asarray(Wk, dtype=np.float32)
    v = np.asarray(v, dtype=np.float32)
    mask = np.asarray(memory_mask)

    keep = ~mask                                   # [B, S]
    counts = keep.sum(1).astype(np.int64)
    order = np.argsort(-counts, kind="stable")
    slot0 = order[:N_CORES]
    slot1 = order[N_CORES:]
    BD = int(counts.max())                 # fixed slot0 boundary (padded)
    mixj = BD // P                         # the one tile straddling batches
    T = math.ceil((BD + int(counts[slot1].max())) / P)
    s_tot = T * P
    chunks = _chunks(T)

    kept_idx = [np.flatnonzero(keep[b]) for b in range(B)]

    def packed_cols(c):
        b0, b1 = int(slot0[c]), int(slot1[c])
        i0, i1 = kept_idx[b0], kept_idx[b1]
        cols = np.empty((s_tot, D), dtype=np.float32)
        cols[:len(i0)] = memory[b0][i0]
        cols[len(i0):BD] = memory[b0][i0[0]]
        cols[BD:BD + len(i1)] = memory[b1][i1]
        cols[BD + len(i1):] = memory[b1][i1[0]]
        return cols

    def chunked_T(W):  # [D, D] -> [P, DC*D] with partition = d % 128
        return np.ascontiguousarray(
            W.T.astype(BF16NP).reshape(DC, P, D).transpose(1, 0, 2).reshape(P, DC * D)
        )

    wkT = chunked_T(Wk)
    wqT = chunked_T(Wq)
    tgtT_full = target.T.astype(BF16NP).reshape(DC, P, B).transpose(1, 0, 2)  # [P, DC, B]
    vbc_arr = np.ascontiguousarray(np.broadcast_to(v.astype(BF16NP), (P, D)))

    # constant selector: block 0 = slot0, block 1 = slot1, block 2 = the
    # mixed boundary tile (first BD-mixj*P positions belong to slot0)
    sel = np.zeros((NB, 3 * P), dtype=np.float32)
    sel[0, 0:P] = 1.0
    sel[1, P:2 * P] = 1.0
    cut = BD - mixj * P
    sel[0, 2 * P:2 * P + cut] = 1.0
    sel[1, 2 * P + cut:3 * P] = 1.0

    pos = (np.arange(T)[None, :] * P + np.arange(P)[:, None])  # [P, T]

    in_maps = []
    for c in range(N_CORES):
        b0, b1 = int(slot0[c]), int(slot1[c])
        n0, n1 = len(kept_idx[b0]), len(kept_idx[b1])
        A = packed_cols(c).T.astype(BF16NP).reshape(DC, P, s_tot).transpose(1, 0, 2)
        blocks = [
            np.ascontiguousarray(A[:, :, off * P:(off + cnt) * P]).reshape(P, DC * cnt * P)
            for (off, cnt) in chunks
        ]
        memC = np.concatenate(blocks, axis=1)
        bm0_arr = (pos < n0).astype(np.float32)
        bm1_arr = ((pos >= BD) & (pos < BD + n1)).astype(np.float32)
        pen_arr = np.where((bm0_arr > 0) | (bm1_arr > 0), 0.0, -1e9).astype(np.float32)
        in_maps.append({
            "memC": memC,
            "wkT": wkT,
            "wqT": wqT,
            "tgtT": np.ascontiguousarray(
                tgtT_full[:, :, [b0, b1]].reshape(P, DC * NB)
            ),
            "vbc": vbc_arr,
            "pen": pen_arr,
            "bm0": bm0_arr,
            "bm1": bm1_arr,
            "sel": sel,
            "head0": np.ascontiguousarray(
                np.concatenate([wkT[:, :D], memC[:, :D]], axis=1)
            ),
        })
    meta = {
        "T": T, "mixj": mixj, "BD": BD,
        "slot0": slot0, "slot1": slot1,
        "counts": counts, "kept_idx": kept_idx,
    }
    return in_maps, meta


def gather_output(results, meta):
    T, BD = meta["T"], meta["BD"]
    out = np.zeros((B, S), dtype=np.float32)
    for c in range(N_CORES):
        arr = np.asarray(results[c]["out"], dtype=np.float32)  # [P, T]
        compact = arr.T.reshape(T * P)  # position j*128+p at [p, j]
        b0, b1 = int(meta["slot0"][c]), int(meta["slot1"][c])
        i0, i1 = meta["kept_idx"][b0], meta["kept_idx"][b1]
        out[b0, i0] = compact[:len(i0)]
        out[b1, i1] = compact[BD:BD + len(i1)]
    return out


def kernel(memory, target, memory_mask, Wq, Wk, v):
    from concourse.bass_utils import run_bass_kernel_spmd

    in_maps, meta = prepare_in_maps(memory, target, memory_mask, Wq, Wk, v)
    nc = get_program(T=meta["T"], mixj=meta["mixj"])
    res = run_bass_kernel_spmd(nc, in_maps, list(range(N_CORES)))
    return gather_output(res.results, meta)


# revision 61
# speedup vs baseline: 1.0403x; 1.0403x over previous
"""Additive (Bahdanau) attention scoring kernel for Trainium2, 8-core SPMD.

Reference computation (B=16, S=4096, D=1024, all fp32):
    q      = target @ Wq.T                    # [B, D]
    k      = memory @ Wk.T                    # [B, S, D]
    scores = tanh(q[:, None, :] + k) @ v      # [B, S]
    out    = softmax(scores - 1e9 * mask, axis=-1)

Sharding: 2 batches per core (host-paired: largest-kept with
smallest-kept), weights replicated.

Host-side prep is layout/quantize only: memory is transposed to [D, S]
per batch, compacted to the unmasked positions (masked positions
contribute exactly 0 to the softmax since exp(-1e9) underflows to 0 in
fp32, so dropping them is algebraically exact), cast to bf16, and
pre-tiled so each DMA is one contiguous block. Both batches of a core
are PACKED into one compact position stream: slot0's kept positions
padded to a fixed boundary BD (the global max kept count), slot1's
appended right after — so the core processes ceil((BD+n1)/128) s-tiles
of 128 positions instead of 2*ceil(BD/128). Exactly one tile (MIXJ)
straddles the batch boundary; a constant 0/1 selector builds its mixed
q-tile, and per-batch masks split the two softmax reductions.

Device layout ("s on partitions"): for each s-tile the PE computes
  k_ps[s=128, e=1024] += memtile[d=128, s=128].T @ WkT[d=128, e]
accumulated over the 8 d-chunks (16 matmuls of N=512, mem stationary,
Wk moving, all bf16). Everything else runs off the PE: DVE adds the
batch's q-tile (materialized once via K=2 selector matmuls against the
on-device q = Wq-matmul result), ACT applies tanh (fp32 PSUM -> bf16
SBUF), and a DVE scalar_tensor_tensor multiplies by the broadcast v
with accum_out (per-partition free-axis sum) yielding the 128 scores.
Finale: add a 0/-1e9 pad-penalty, ACT Exp, two masked DVE reductions
(one per batch), two 128x128 ones-matmuls to reduce+broadcast the
totals, then one fused scale pass and a single compact DMA out. The
host scatters to full S (pure indexing; masked positions are exactly
0).
"""

import math
import os
from contextlib import ExitStack

import ml_dtypes
import numpy as np

import concourse.tile as tile
from concourse import bacc, mybir
import concourse.bass as bass  # noqa: F401

B, S, D = 16, 4096, 1024
N_CORES = 8
NB = B // N_CORES  # batches per core
P = 128
DC = D // P        # contraction chunks
EH = D // 512      # moving-operand halves (PSUM bank = 512 fp32)

F32 = mybir.dt.float32
BF16 = mybir.dt.bfloat16
AF = mybir.ActivationFunctionType
MUL = mybir.AluOpType.mult
ADD = mybir.AluOpType.add

BF16NP = ml_dtypes.bfloat16

_CACHE = {}


def _chunks(T):
    """DMA chunks of up to 4 s-tiles (1 MiB of bf16 per full chunk). The
    first chunk is a single tile so the PE's first matmul dependency is a
    256 KiB transfer."""
    return [(0, 1)] + [(i, min(4, T - i)) for i in range(1, T, 4)]


def _build_program(T, mixj, stage):
    chunks = _chunks(T)

    nc = bacc.Bacc("TRN2", target_bir_lowering=False, debug=False)

    memC = nc.dram_tensor("memC", [P, DC * T * P], BF16, kind="ExternalInput").ap()
    wkT = nc.dram_tensor("wkT", [P, DC * D], BF16, kind="ExternalInput").ap()
    wqT = nc.dram_tensor("wqT", [P, DC * D], BF16, kind="ExternalInput").ap()
    tgtT = nc.dram_tensor("tgtT", [P, DC * NB], BF16, kind="ExternalInput").ap()
    vbc = nc.dram_tensor("vbc", [P, D], BF16, kind="ExternalInput").ap()
    pen = nc.dram_tensor("pen", [P, T], F32, kind="ExternalInput").ap()
    bm0 = nc.dram_tensor("bm0", [P, T], F32, kind="ExternalInput").ap()
    bm1 = nc.dram_tensor("bm1", [P, T], F32, kind="ExternalInput").ap()
    sel = nc.dram_tensor("sel", [NB, 3 * P], F32, kind="ExternalInput").ap()
    # head0 = [Wk d-chunk 0 | memC tile 0] packed so one completion wait
    # gates the first real matmul
    head0 = nc.dram_tensor("head0", [P, 2 * D], BF16, kind="ExternalInput").ap()
    out = nc.dram_tensor("out", [P, T], F32, kind="ExternalOutput").ap()

    with tile.TileContext(nc) as tc, ExitStack() as ctx:
        consts = ctx.enter_context(tc.tile_pool(name="consts", bufs=1))
        mem_pool = ctx.enter_context(tc.tile_pool(name="mem", bufs=3))
        ti_pool = ctx.enter_context(tc.tile_pool(name="ti", bufs=3))
        tt_pool = ctx.enter_context(tc.tile_pool(name="tt", bufs=3))
        ttv_pool = ctx.enter_context(tc.tile_pool(name="ttv", bufs=2))
        fin_pool = ctx.enter_context(tc.tile_pool(name="fin", bufs=2))
        kps_pool = ctx.enter_context(tc.tile_pool(name="kps", bufs=3, space="PSUM"))
        qps_pool = ctx.enter_context(tc.tile_pool(name="qps", bufs=1, space="PSUM"))

        # DMA plumbing: the per-DMA completion cost serializes per HWDGE
        # ring, so head-critical transfers are split across both rings.
        head_sb = consts.tile([P, 2 * D], BF16)
        nc.scalar.dma_start(head_sb[:], head0[:, :])
        w1 = chunks[1][1] * P
        mem1_sb = mem_pool.tile([P, DC * 4 * P], BF16, tag="mem", name="mem_sb")
        nc.scalar.dma_start(
            mem1_sb[:, :DC * w1],
            memC[:, DC * chunks[1][0] * P: DC * chunks[1][0] * P + DC * w1],
        )
        # Wk chunks pace the lead k-matmuls on sync's ring; Wq follows them
        # on the SAME ring so later memC chunk DMAs (emitted in-loop, hence
        # behind Wq in this ring's FIFO) cannot steal its HBM bandwidth —
        # the q-matmuls stalled ~2.8us on late Wq when it rode the other
        # ring against the memC stream.
        wk_sb = consts.tile([P, DC * D], BF16)
        for dc in range(DC):
            nc.sync.dma_start(
                wk_sb[:, dc * D:(dc + 1) * D], wkT[:, dc * D:(dc + 1) * D]
            )
        wq_sb = consts.tile([P, DC * D], BF16)
        nc.sync.dma_start(wq_sb[:], wqT[:, :])
        tgt_sb = consts.tile([P, DC * NB], BF16)
        nc.scalar.dma_start(tgt_sb[:], tgtT[:, :])
        sel_sb = consts.tile([NB, 3 * P], F32)
        nc.scalar.dma_start(sel_sb[:], sel[:, :])
        vbc_sb = consts.tile([P, D], BF16)
        nc.scalar.dma_start(vbc_sb[:], vbc[:, :])
        pen_sb = consts.tile([P, T], F32)
        nc.scalar.dma_start(pen_sb[:], pen[:, :])
        bm0_sb = consts.tile([P, T], F32)
        nc.scalar.dma_start(bm0_sb[:], bm0[:, :])
        bm1_sb = consts.tile([P, T], F32)
        nc.scalar.dma_start(bm1_sb[:], bm1[:, :])

        ones128 = consts.tile([P, P], F32)
        nc.vector.memset(ones128[:], 1.0)

        # PE pre-warm with VARYING data (HAM appears to watch switching
        # activity, not busy cycles — all-zero dummies left it cold): iota
        # operands make the array toggle so the clock gate reaches 8/8
        # before the real k-matmuls arrive.
        io_sb = consts.tile([P, 512], F32)
        nc.gpsimd.iota(io_sb[:], pattern=[[1, 512]], base=1, channel_multiplier=3,
                       allow_small_or_imprecise_dtypes=True)
        warm_ps = qps_pool.tile([P, D], F32, tag="qps", name="warm_ps")
        for _ in range(5):
            nc.tensor.matmul(warm_ps[:, 0:512], io_sb[:, 0:P], io_sb[:], start=True, stop=True)

        q_sb = consts.tile([NB, D], F32)
        qt_sb = consts.tile([P, 3 * D], F32)
        score_sb = consts.tile([P, T], F32)
        rs_sbs = [consts.tile([P, 1], F32, tag=f"rs{i}", name=f"rs{i}") for i in range(2)]
        rc_sbs = [consts.tile([P, 1], F32, tag=f"rc{i}", name=f"rc{i}") for i in range(2)]

        def emit_qsetup():
            # q = target @ Wq.T on the PE, fp32 accumulate -> [NB, D]
            q_ps = qps_pool.tile([P, D], F32, tag="qps", name="q_ps")
            for eh in range(EH):
                for dc in range(DC):
                    nc.tensor.matmul(
                        q_ps[0:NB, eh * 512:(eh + 1) * 512],
                        tgt_sb[:, dc * NB:(dc + 1) * NB],
                        wq_sb[:, dc * D + eh * 512: dc * D + (eh + 1) * 512],
                        start=(dc == 0),
                        stop=(dc == DC - 1),
                    )
            nc.vector.tensor_copy(q_sb[:], q_ps[0:NB, :])
            # q-tiles (slot0 / slot1 / mixed boundary tile) via K=2 selector
            # matmuls: qt[i][s, e] = sum_b sel[b, i*P+s] * q[b, e]
            for i in range(3):
                qt_ps = qps_pool.tile([P, D], F32, tag="qps", name="qt_ps")
                for eh in range(EH):
                    nc.tensor.matmul(
                        qt_ps[:, eh * 512:(eh + 1) * 512],
                        sel_sb[:, i * P:(i + 1) * P],
                        q_sb[0:NB, eh * 512:(eh + 1) * 512],
                        start=True,
                        stop=True,
                    )
                nc.vector.tensor_copy(qt_sb[:, i * D:(i + 1) * D], qt_ps[:])

        def qt_slice(j):
            i = 0 if j < mixj else (2 if j == mixj else 1)
            return qt_sb[:, i * D:(i + 1) * D]

        def emit_tile0_mm():
            # tile 0 reads both its stationary (mem tile 0) and the dc=0
            # moving chunk from the packed head0 transfer. eh-outer order:
            # 8 consecutive matmuls per PSUM bank instead of alternating
            # banks every matmul (bank cycling triggers HAM oscillation).
            k_ps = kps_pool.tile([P, D], F32, tag="kps", name="k_ps")
            for dc in range(DC):
                for eh in range(EH):
                    mv = (head_sb[:, eh * 512:(eh + 1) * 512] if dc == 0
                          else wk_sb[:, dc * D + eh * 512: dc * D + (eh + 1) * 512])
                    nc.tensor.matmul(
                        k_ps[:, eh * 512:(eh + 1) * 512],
                        head_sb[:, D + dc * P: D + (dc + 1) * P],
                        mv,
                        start=(dc == 0),
                        stop=(dc == DC - 1),
                    )
            return k_ps

        def emit_tile_mm(mem_sb, w, t):
            k_ps = kps_pool.tile([P, D], F32, tag="kps", name="k_ps")
            for dc in range(DC):
                for eh in range(EH):
                    nc.tensor.matmul(
                        k_ps[:, eh * 512:(eh + 1) * 512],
                        mem_sb[:, dc * w + t * P: dc * w + (t + 1) * P],
                        wk_sb[:, dc * D + eh * 512: dc * D + (eh + 1) * 512],
                        start=(dc == 0),
                        stop=(dc == DC - 1),
                    )
            return k_ps

        def emit_tile_post(k_ps, j, split=False):
            ti = ti_pool.tile([P, D], F32, tag="ti", name="ti")
            tt = tt_pool.tile([P, D], BF16, tag="tt", name="tt")
            ttv = ttv_pool.tile([P, D], BF16, tag="ttv", name="ttv")
            if not split:
                nc.vector.tensor_add(ti[:], k_ps[:], qt_slice(j))
                nc.scalar.activation(tt[:], ti[:], AF.Tanh)
                nc.vector.scalar_tensor_tensor(
                    ttv[:], tt[:], 1.0, vbc_sb[:],
                    op0=MUL, op1=MUL,
                    accum_out=score_sb[:, j:j + 1],
                )
                return
            # tail latency: run the last tile's post chain in two pipelined
            # halves so the end-of-kernel serial chain is shorter
            qs = qt_slice(j)
            sc = consts.tile([P, 2], F32, tag="scsplit", name="scsplit")
            for h in range(2):
                hs = slice(h * 512, (h + 1) * 512)
                nc.vector.tensor_add(ti[:, hs], k_ps[:, hs], qs[:, hs])
                nc.scalar.activation(tt[:, hs], ti[:, hs], AF.Tanh)
                nc.vector.scalar_tensor_tensor(
                    ttv[:, hs], tt[:, hs], 1.0, vbc_sb[:, hs],
                    op0=MUL, op1=MUL,
                    accum_out=sc[:, h:h + 1],
                )
            nc.vector.tensor_add(score_sb[:, j:j + 1], sc[:, 0:1], sc[:, 1:2])

        # --- main loop over the packed s-tiles. The first 3 tiles'
        # k-matmuls (= kps pool depth) are emitted before the q-setup so
        # the PE can start as soon as head0 lands; their post-processing
        # (which needs the q-tiles) follows it. slot0's exp + masked
        # reduction (bm0 is zero past the boundary tile) runs mid-stream
        # right after the boundary tile, keeping the end-of-kernel chain
        # to slot1 only. ---
        sm = consts.tile([P, T], F32)
        ex = consts.tile([P, T], F32)
        c0 = mixj + 1  # columns 0:c0 cover every bm0-nonzero position
        ems = [fin_pool.tile([P, T], F32, tag=f"em{i}", name=f"em{i}") for i in range(2)]

        st0 = fin_pool.tile([P, T], F32, tag="st0", name="st0")

        def emit_fin0():
            nc.vector.tensor_add(sm[:, 0:c0], score_sb[:, 0:c0], pen_sb[:, 0:c0])
            if stage >= 2:
                nc.scalar.activation(ex[:, 0:c0], sm[:, 0:c0], AF.Exp)
                nc.vector.scalar_tensor_tensor(
                    ems[0][:, 0:c0], ex[:, 0:c0], 1.0, bm0_sb[:, 0:c0],
                    op0=MUL, op1=MUL, accum_out=rs_sbs[0][:],
                )

        def emit_fin0_back():
            # slot0's totals matmul + scale + its slice of the output DMA,
            # emitted a few tiles after fin0 so the PE has dependency slack
            # — only slot1's chain remains after the last k-matmul
            tot_ps = qps_pool.tile([P, D], F32, tag="qps", name="tot_ps")
            nc.tensor.matmul(tot_ps[:, 0:1], ones128[:], rs_sbs[0][:], start=True, stop=True)
            nc.vector.reciprocal(rc_sbs[0][:], tot_ps[:, 0:1])
            nc.vector.tensor_scalar_mul(st0[:, 0:c0], ems[0][:, 0:c0], rc_sbs[0][:])
            nc.sync.dma_start(out[:, 0:mixj], st0[:, 0:mixj])

        done_fin0 = False
        for ci, (coff, cnt) in enumerate(chunks):
            w = cnt * P
            if ci == 0:
                lead0 = emit_tile0_mm()
                continue
            if ci == 1:
                mem_sb = mem1_sb
            else:
                mem_sb = mem_pool.tile([P, DC * 4 * P], BF16, tag="mem", name="mem_sb")
                nc.sync.dma_start(
                    mem_sb[:, :DC * w],
                    memC[:, DC * coff * P: DC * coff * P + DC * w],
                )
            if ci == 1:
                leads = [emit_tile_mm(mem_sb, w, t) for t in range(2)]
                emit_qsetup()
                emit_tile_post(lead0, 0)
                for t in range(2):
                    emit_tile_post(leads[t], coff + t)
                for t in range(2, cnt):
                    emit_tile_post(emit_tile_mm(mem_sb, w, t), coff + t)
                continue
            last = ci == len(chunks) - 1
            for t in range(cnt):
                j = coff + t
                emit_tile_post(
                    emit_tile_mm(mem_sb, w, t), j,
                    split=(last and t == cnt - 1),
                )
                if j == mixj:
                    emit_fin0()
                    done_fin0 = True
                if j == mixj + 3 and stage >= 2:
                    emit_fin0_back()
        if not done_fin0:
            emit_fin0()
            if stage >= 2:
                emit_fin0_back()

        # --- finale back half: slot1's chain + the shared scale/output ---
        nc.vector.tensor_add(sm[:, c0:T], score_sb[:, c0:T], pen_sb[:, c0:T])
        if stage < 2:
            nc.sync.dma_start(out[:, :], sm[:])
        else:
            nc.scalar.activation(ex[:, c0:T], sm[:, c0:T], AF.Exp)
            nc.vector.scalar_tensor_tensor(
                ems[1][:, mixj:T], ex[:, mixj:T], 1.0, bm1_sb[:, mixj:T],
                op0=MUL, op1=MUL, accum_out=rs_sbs[1][:],
            )
            tot_ps = qps_pool.tile([P, D], F32, tag="qps", name="tot_ps")
            nc.tensor.matmul(tot_ps[:, 0:1], ones128[:], rs_sbs[1][:], start=True, stop=True)
            nc.vector.reciprocal(rc_sbs[1][:], tot_ps[:, 0:1])
            # out = em0*r0 + em1*r1 (em_i are already the masked exps);
            # em0 is defined on cols 0:c0, em1 on cols mixj:T — only the
            # boundary column mixj needs both terms. Cols 0:mixj went out
            # with emit_fin0_back already.
            ot = fin_pool.tile([P, T], F32, tag="ot", name="ot")
            nc.vector.scalar_tensor_tensor(
                ot[:, mixj:c0], ems[1][:, mixj:c0], rc_sbs[1][:], st0[:, mixj:c0],
                op0=MUL, op1=ADD,
            )
            nc.vector.tensor_scalar_mul(ot[:, c0:T], ems[1][:, c0:T], rc_sbs[1][:])
            nc.sync.dma_start(out[:, mixj:T], ot[:, mixj:T])

    nc.compile()
    return nc


def get_program(T=None, mixj=None, stage=None):
    if stage is None:
        stage = int(os.environ.get("KERNEL_STAGE", "2"))
    key = (T, mixj, stage)
    if key not in _CACHE:
        _CACHE[key] = _build_program(T, mixj, stage)
    return _CACHE[key]


def prepare_in_maps(memory, target, memory_mask, Wq, Wk, v):
    memory = np.asarray(memory, dtype=np.float32)
    target = np.asarray(target, dtype=np.float32)
    Wq = np.asarray(Wq, dtype=np.float32)
    Wk = np.asarray(Wk, dtype=np.float32)
    v = np.asarray(v, dtype=np.float32)
    mask = np.asarray(memory_mask)

    keep = ~mask                                   # [B, S]
    counts = keep.sum(1).astype(np.int64)
    order = np.argsort(-counts, kind="stable")
    slot0 = order[:N_CORES]
    slot1 = order[N_CORES:]
    BD = int(counts.max())                 # fixed slot0 boundary (padded)
    mixj = BD // P                         # the one tile straddling batches
    T = math.ceil((BD + int(counts[slot1].max())) / P)
    s_tot = T * P
    chunks = _chunks(T)

    kept_idx = [np.flatnonzero(keep[b]) for b in range(B)]

    def packed_cols(c):
        b0, b1 = int(slot0[c]), int(slot1[c])
        i0, i1 = kept_idx[b0], kept_idx[b1]
        cols = np.empty((s_tot, D), dtype=np.float32)
        cols[:len(i0)] = memory[b0][i0]
        cols[len(i0):BD] = memory[b0][i0[0]]
        cols[BD:BD + len(i1)] = memory[b1][i1]
        cols[BD + len(i1):] = memory[b1][i1[0]]
        return cols

    def chunked_T(W):  # [D, D] -> [P, DC*D] with partition = d % 128
        return np.ascontiguousarray(
            W.T.astype(BF16NP).reshape(DC, P, D).transpose(1, 0, 2).reshape(P, DC * D)
        )

    wkT = chunked_T(Wk)
    wqT = chunked_T(Wq)
    tgtT_full = target.T.astype(BF16NP).reshape(DC, P, B).transpose(1, 0, 2)  # [P, DC, B]
    vbc_arr = np.ascontiguousarray(np.broadcast_to(v.astype(BF16NP), (P, D)))

    # constant selector: block 0 = slot0, block 1 = slot1, block 2 = the
    # mixed boundary tile (first BD-mixj*P positions belong to slot0)
    sel = np.zeros((NB, 3 * P), dtype=np.float32)
    sel[0, 0:P] = 1.0
    sel[1, P:2 * P] = 1.0
    cut = BD - mixj * P
    sel[0, 2 * P:2 * P + cut] = 1.0
    sel[1, 2 * P + cut:3 * P] = 1.0

    pos = (np.arange(T)[None, :] * P + np.arange(P)[:, None])  # [P, T]

    in_maps = []
    for c in range(N_CORES):
        b0, b1 = int(slot0[c]), int(slot1[c])
        n0, n1 = len(kept_idx[b0]), len(kept_idx[b1])
        A = packed_cols(c).T.astype(BF16NP).reshape(DC, P, s_tot).transpose(1, 0, 2)
        blocks = [
            np.ascontiguousarray(A[:, :, off * P:(off + cnt) * P]).reshape(P, DC * cnt * P)
            for (off, cnt) in chunks
        ]
        memC = np.concatenate(blocks, axis=1)
        bm0_arr = (pos < n0).astype(np.float32)
        bm1_arr = ((pos >= BD) & (pos < BD + n1)).astype(np.float32)
        pen_arr = np.where((bm0_arr > 0) | (bm1_arr > 0), 0.0, -1e9).astype(np.float32)
        in_maps.append({
            "memC": memC,
            "wkT": wkT,
            "wqT": wqT,
            "tgtT": np.ascontiguousarray(
                tgtT_full[:, :, [b0, b1]].reshape(P, DC * NB)
            ),
            "vbc": vbc_arr,
            "pen": pen_arr,
            "bm0": bm0_arr,
            "bm1": bm1_arr,
            "sel": sel,
            "head0": np.ascontiguousarray(
                np.concatenate([wkT[:, :D], memC[:, :D]], axis=1)
            ),
        })
    meta = {
        "T": T, "mixj": mixj, "BD": BD,
        "slot0": slot0, "slot1": slot1,
        "counts": counts, "kept_idx": kept_idx,
    }
    return in_maps, meta


def gather_output(results, meta):
    T, BD = meta["T"], meta["BD"]
    out = np.zeros((B, S), dtype=np.float32)
    for c in range(N_CORES):
        arr = np.asarray(results[c]["out"], dtype=np.float32)  # [P, T]
        compact = arr.T.reshape(T * P)  # position j*128+p at [p, j]
        b0, b1 = int(meta["slot0"][c]), int(meta["slot1"][c])
        i0, i1 = meta["kept_idx"][b0], meta["kept_idx"][b1]
        out[b0, i0] = compact[:len(i0)]
        out[b1, i1] = compact[BD:BD + len(i1)]
    return out


def kernel(memory, target, memory_mask, Wq, Wk, v):
    from concourse.bass_utils import run_bass_kernel_spmd

    in_maps, meta = prepare_in_maps(memory, target, memory_mask, Wq, Wk, v)
    nc = get_program(T=meta["T"], mixj=meta["mixj"])
    res = run_bass_kernel_spmd(nc, in_maps, list(range(N_CORES)))
    return gather_output(res.results, meta)
